# revision 6
# baseline (speedup 1.0000x reference)
"""v9: sigma-tuned Nc=20 basis (sb=0.065, minimax fit), bf16 feature-major
tiles [128, Nc, CH], merged/batched ACT exps, ratio-doubling chain on DVE,
per-chunk quadrant-rotated matmuls kept back-to-back across all diagrams
(tails batched at the end), gpsimd only for off-critical-path arg builds."""

import numpy as np
from contextlib import ExitStack

import concourse.bass as bass
import concourse.bacc as bacc
import concourse.tile as tile
from concourse import mybir

F32 = mybir.dt.float32
BF16 = mybir.dt.bfloat16

RESOLUTION = 50
SIGMA = 0.05
NF = float(np.float64(1.0) / (2.0 * SIGMA**2 + 1e-8))
SB = 0.065                    # basis gaussian sigma (wider than target)
NFB = float(1.0 / (2.0 * SB * SB))
SQNFB = float(np.sqrt(NFB))
MARGIN = 0.05
NC = 20
NJOBS = 2
JC = NC // NJOBS              # 10


def make_host_constants():
    """Minimax-fit basis weights with kappa folding; per-job chain consts."""
    bs = np.linspace(0.0, 1.0, 4001)
    x = np.linspace(0.0, 1.0, RESOLUTION)
    G = np.exp(-NF * (x[None, :] - bs[:, None]) ** 2)
    xc = np.linspace(-MARGIN, 1.0 + MARGIN, NC)
    Phi = np.exp(-NFB * (xc[None, :] - bs[:, None]) ** 2)
    wt = np.ones(len(bs))
    best = None
    for _ in range(26):
        Pw = Phi * wt[:, None]
        W = np.linalg.solve(Phi.T @ Pw + 1e-11 * np.eye(NC), Pw.T @ G)
        R = Phi @ W - G
        m = np.abs(R).max()
        if best is None or m < best[0]:
            best = (m, W.copy())
        resid = np.abs(R).max(axis=1)
        wt = wt * (0.1 + resid / resid.max())
        wt /= wt.mean()
    W = best[1]
    h = float(xc[1] - xc[0])
    Wt = W.copy()
    xc0s, cJs = [], []
    for ji in range(NJOBS):
        J = np.arange(ji * JC, (ji + 1) * JC)
        xc0 = float(xc[J[0]])
        cJ = 0.5 * (float(xc[J[0]]) + float(xc[J[-1]]))
        u = xc[J] - cJ
        for k, c in enumerate(J):
            kappa = NFB * (u[k] ** 2 - u[0] ** 2)
            Wt[c, :] = W[c, :] * np.exp(-kappa)
        xc0s.append(xc0)
        cJs.append(cJ)
    return xc0s, cJs, h, Wt.astype(np.float32)


def build_kernel(DG=4, N=65536, debug=False):
    assert N % 128 == 0
    CH = N // 128                        # 512
    xc0s, cJs, h, Wt = make_host_constants()

    nc = bacc.Bacc("TRN2", target_bir_lowering=False, debug=debug)

    diagrams = nc.declare_dram_parameter("diagrams", [DG, N, 2], F32, isOutput=False)
    wtx_d = nc.declare_dram_parameter("wtx", [NC, RESOLUTION], F32, isOutput=False)
    wty_d = nc.declare_dram_parameter("wty", [NC, RESOLUTION], F32, isOutput=False)
    out_d = nc.declare_dram_parameter("out", [DG, RESOLUTION, RESOLUTION], F32, isOutput=True)

    with ExitStack() as ctx:
        tc = ctx.enter_context(tile.TileContext(nc))
        singles = ctx.enter_context(tc.tile_pool(name="singles", bufs=1))
        raws = ctx.enter_context(tc.tile_pool(name="raws", bufs=2))
        args = ctx.enter_context(tc.tile_pool(name="args", bufs=2))
        rats = ctx.enter_context(tc.tile_pool(name="rats", bufs=2))
        smalls = ctx.enter_context(tc.tile_pool(name="smalls", bufs=2))
        tpool = ctx.enter_context(tc.tile_pool(name="tpool", bufs=2))
        psums = ctx.enter_context(tc.tile_pool(name="psums", bufs=1, space="PSUM"))
        psum2 = ctx.enter_context(tc.tile_pool(name="psum2", bufs=2, space="PSUM"))
        outs = ctx.enter_context(tc.tile_pool(name="outs", bufs=2))

        # per-job Square bias tiles: -SQNFB * xc0_j
        bias_t = []
        for ji in range(NJOBS):
            bt = singles.tile([128, 1], F32, tag=f"bias{ji}", name=f"bias{ji}")
            nc.vector.memset(bt[:], float(-SQNFB * xc0s[ji]))
            bias_t.append(bt)

        wtx_t = singles.tile([NC, RESOLUTION], F32)
        nc.sync.dma_start(out=wtx_t[:], in_=wtx_d[:])
        wty_t = singles.tile([NC, RESOLUTION], F32)
        nc.sync.dma_start(out=wty_t[:], in_=wty_d[:])

        EXP = mybir.ActivationFunctionType.Exp
        SQU = mybir.ActivationFunctionType.Square
        MUL = mybir.AluOpType.mult
        ADD = mybir.AluOpType.add

        def t_ap(T, ax, f0, nf):
            """AP over T rows {f0..f0+nf-1} and {JC+f0..} of axis ax."""
            base = T[:]
            off = base.offset + (ax * NC + f0) * CH
            if nf == 1:
                ap = [base.ap[0], [JC * CH, 2], [1, CH]]
            else:
                ap = [base.ap[0], [JC * CH, 2], [CH, nf], [1, CH]]
            return bass.AP(tensor=base.tensor, offset=off, ap=ap)

        def emit_prep(dg):
            raw = raws.tile([128, CH * 2], F32, tag="raw", name=f"raw{dg}")
            dsrc = diagrams[dg].rearrange("(p c) t -> p (c t)", p=128)
            for si in range(4):
                sl = slice(si * CH * 2 // 4, (si + 1) * CH * 2 // 4)
                nc.sync.dma_start(out=raw[:, sl], in_=dsrc[:, sl])
            raw3 = raw.rearrange("p (c t) -> p c t", t=2)
            b_ap = raw3[:, :, 0]
            d_ap = raw3[:, :, 1]

            T = tpool.tile([128, 2, NC, CH], BF16, tag="T", name=f"T{dg}")
            argb = args.tile([128, NJOBS, CH, 2], F32, tag="argb", name=f"argb{dg}")
            rarg = args.tile([128, 2, NJOBS, CH], F32, tag="rarg", name=f"rarg{dg}")
            r1 = rats.tile([128, 2, NJOBS, CH], BF16, tag="r1", name=f"r1{dg}")
            r2 = rats.tile([128, 2, NJOBS, CH], BF16, tag="r2", name=f"r2{dg}")
            r4 = rats.tile([128, 2, NJOBS, CH], BF16, tag="r4", name=f"r4{dg}")
            xseed = smalls.tile([128, NJOBS, CH], BF16, tag="xseed", name=f"xseed{dg}")
            pw = smalls.tile([128, CH], F32, tag="pw", name=f"pw{dg}")
            w_t = smalls.tile([128, CH], BF16, tag="w", name=f"w{dg}")

            # gpsimd: ratio args (off critical path; consumed by r1 exp)
            for ax, src in ((0, b_ap), (1, d_ap)):
                for ji in range(NJOBS):
                    nc.gpsimd.tensor_scalar(
                        out=rarg[:, ax, ji], in0=src,
                        scalar1=float(-2.0 * NFB * h), scalar2=float(2.0 * NFB * h * cJs[ji]),
                        op0=MUL, op1=ADD,
                    )
            # DVE: persistence (critical path of w -> f0x)
            nc.vector.tensor_sub(pw[:], d_ap, b_ap)

            # ACT: seed args (both axes at once: raw is (c,t)-interleaved)
            for ji in range(NJOBS):
                nc.scalar.activation(
                    out=argb[:, ji], in_=raw3[:, :, :],
                    func=SQU, scale=SQNFB, bias=bias_t[ji][:],
                )
            # ACT: seed exps, merged across jobs: x -> xseed, y -> T rows {0, JC}
            ab = argb[:]
            in_x = bass.AP(tensor=ab.tensor, offset=ab.offset,
                           ap=[ab.ap[0], [CH * 2, 2], [2, CH]])
            in_y = bass.AP(tensor=ab.tensor, offset=ab.offset + 1,
                           ap=[ab.ap[0], [CH * 2, 2], [2, CH]])
            nc.scalar.activation(out=xseed[:], in_=in_x, func=EXP, scale=-1.0)
            # ACT: w = pw^2 (bf16 out) -- early: unblocks f0x
            nc.scalar.activation(out=w_t[:], in_=pw[:], func=SQU, scale=1.0)
            nc.scalar.activation(out=t_ap(T, 1, 0, 1), in_=in_y, func=EXP, scale=-1.0)
            # ACT: r1 = exp(-rarg), all axes+jobs in one op
            nc.scalar.activation(out=r1[:], in_=rarg[:], func=EXP, scale=-1.0)

            # DVE: f0x = xseed * w -> T[x] rows {0, JC}
            wb = w_t[:]
            w_bc = bass.AP(tensor=wb.tensor, offset=wb.offset,
                           ap=[wb.ap[0], [0, 2], [1, CH]])
            nc.vector.tensor_mul(t_ap(T, 0, 0, 1), xseed[:], w_bc)
            # DVE: r2 = r1^2, r4 = r2^2 (both axes at once)
            nc.vector.tensor_mul(r2[:], r1[:], r1[:])
            nc.vector.tensor_mul(r4[:], r2[:], r2[:])

            for ax in (0, 1):
                rr1, rr2, rr4 = r1[:, ax], r2[:, ax], r4[:, ax]

                def rbc(rr, nf):
                    if nf == 1:
                        ap = [rr.ap[0], [CH, 2], [1, CH]]
                    else:
                        ap = [rr.ap[0], [CH, 2], [0, nf], [1, CH]]
                    return bass.AP(tensor=rr.tensor, offset=rr.offset, ap=ap)

                nc.vector.tensor_mul(t_ap(T, ax, 1, 1), t_ap(T, ax, 0, 1), rbc(rr1, 1))
                nc.vector.tensor_mul(t_ap(T, ax, 2, 2), t_ap(T, ax, 0, 2), rbc(rr2, 2))
                nc.vector.tensor_mul(t_ap(T, ax, 4, 4), t_ap(T, ax, 0, 4), rbc(rr4, 4))
                nc.vector.tensor_mul(t_ap(T, ax, 8, 2), t_ap(T, ax, 4, 2), rbc(rr4, 2))
            return T

        def emit_mms(dg, T, hp):
            base = T[:]
            for c in range(CH):
                q = c % 4
                lhs = bass.AP(tensor=base.tensor, offset=base.offset + c,
                              ap=[base.ap[0], [CH, NC]])
                rhs = bass.AP(tensor=base.tensor, offset=base.offset + NC * CH + c,
                              ap=[base.ap[0], [CH, NC]])
                nc.tensor.matmul(
                    hp[32 * q:32 * q + NC, 0:NC], lhs, rhs,
                    start=(c == q), stop=(c == CH - 4 + q),
                    tile_position=(0, 32 * q),
                    skip_group_check=True,
                )

        def emit_tail(dg, hp):
            hs = outs.tile([NC, NC], F32, tag="hs", name=f"hs{dg}")
            nc.vector.tensor_copy(hs[:], hp[0:NC, 0:NC])
            for q in range(1, 4):
                nc.vector.tensor_add(hs[:], hs[:], hp[32 * q:32 * q + NC, 0:NC])
            p1 = psum2.tile([NC, RESOLUTION], F32, tag="p1", name=f"p1{dg}")
            nc.tensor.matmul(p1[:], hs[:], wtx_t[:], start=True, stop=True)
            o1 = outs.tile([NC, RESOLUTION], F32, tag="o1", name=f"o1{dg}")
            nc.vector.tensor_copy(o1[:], p1[:])
            p2 = psum2.tile([RESOLUTION, RESOLUTION], F32, tag="p2", name=f"p2{dg}")
            nc.tensor.matmul(p2[:], o1[:], wty_t[:], start=True, stop=True)
            o2 = outs.tile([RESOLUTION, RESOLUTION], F32, tag="o2", name=f"o2{dg}")
            nc.vector.tensor_copy(o2[:], p2[:])
            nc.sync.dma_start(out=out_d[dg], in_=o2[:])

        hps = []
        T = emit_prep(0)
        for dg in range(DG):
            hp = psums.tile([128, 32], F32, tag=f"H{dg}", name=f"H{dg}")
            hps.append(hp)
            emit_mms(dg, T, hp)
            if dg + 1 < DG:
                T = emit_prep(dg + 1)
        for dg in range(DG):
            emit_tail(dg, hps[dg])

    nc.compile()
    return nc, {"wtx": Wt.copy(), "wty": Wt.copy()}


_CACHE = {}


def _get_built():
    if "k" not in _CACHE:
        _CACHE["k"] = build_kernel(DG=4, N=65536)
    return _CACHE["k"]


def kernel(diagrams):
    """Full-input entry point: diagrams [32, 65536, 2] fp32 -> [32, 50, 50] fp32.

    Shards the batch axis over 8 NeuronCores (4 diagrams each), runs the
    Bass kernel SPMD, gathers per-core outputs.
    """
    from concourse.bass_utils import run_bass_kernel_spmd

    diagrams = np.ascontiguousarray(np.asarray(diagrams, dtype=np.float32))
    B, N, two = diagrams.shape
    assert (B, N, two) == (32, 65536, 2), (B, N, two)
    nc, consts = _get_built()
    in_maps = []
    for core in range(8):
        m = {"diagrams": diagrams[core * 4:(core + 1) * 4]}
        m.update(consts)
        in_maps.append(m)
    res = run_bass_kernel_spmd(nc, in_maps, core_ids=list(range(8)))
    out = np.concatenate([res.results[c]["out"] for c in range(8)], axis=0)
    return out.astype(np.float32)
